# revision 1
# baseline (speedup 1.0000x reference)
"""Trainium2 Bass kernel for nn_CrossAttention (sparse_attention).

Cross-attention with three branches (prompt L=77, image L=257, action L=64),
B=8, LQ=4096, D=1024, 16 heads x 64. Sharding: data-parallel over batch —
one batch element per NeuronCore, no collectives.

Per-core dataflow (all in transposed "feature-major" layout so that every
matmul contracts over the SBUF partition dim):
  xT  = transpose(x)                      (PE transposes, bf16)
  qT  = Wq^T @ xT                         (lhsT = Wq as stored)
  kTb = Wb^T @ ctxT, vb = ctx @ Wb        (per branch)
  per head h, branch b:  sT = kT_h^T qT_h;  p = exp(sT*scale)  (no max-sub,
    |s| <= ~3.2 for these inputs);  PV uses v' with a ones column so one
    matmul gives both o^T and the softmax denominator l; branch gates are
    folded into v' at build time.  Normalize: 1/l on DVE, row-broadcast via
    a step-0-free-dim DMA, multiply on DVE, branch-accumulate on gpsimd
    (engine balance: PE ~656us, ACT ~628, DVE ~540, Pool ~285 per core;
    PV outputs borrow the projection psum pool for every third head so the
    normalize chain has 5 banks of depth; cost-model e2e ~1.05ms/core vs
    the ~0.66ms bf16 PE roofline).
  acc = sum of gated branch outputs (still transposed) -> feeds Wq_a and
    Wout projections directly as lhsT/rhs without further transposes.
"""

import numpy as np

H = 16
DH = 64
D = 1024
LQ = 4096
B = 8
NCORES = 8
SCALE = DH ** -0.5
QT = 512           # queries per tile
NQT = LQ // QT     # 8 q-tiles

BRANCHES = {
    "p": 77,
    "i": 257,
    "a": 64,
}


def _chunks(L):
    out = []
    c0 = 0
    while c0 < L:
        out.append((c0, min(128, L - c0)))
        c0 += 128
    return out


def _build_nc(gate_i: float, gate_a: float):
    from contextlib import ExitStack

    import concourse.bass as bass
    import concourse.mybir as mybir
    import concourse.tile as tile
    from concourse import bacc
    from concourse.masks import make_identity

    f32 = mybir.dt.float32
    bf16 = mybir.dt.bfloat16
    AF = mybir.ActivationFunctionType
    ALU = mybir.AluOpType

    # Bacc (not raw Bass): its finalize() runs generate_event_semaphores
    # (walrus allows at most one sync wait per instruction), register
    # allocation, and ISA lowering.
    nc = bacc.Bacc("TRN2", target_bir_lowering=False, debug=False)

    x_d = nc.dram_tensor("x", [LQ, D], f32, kind="ExternalInput")
    ctx_d = {
        "p": nc.dram_tensor("ctx_prompt", [77, D], f32, kind="ExternalInput"),
        "i": nc.dram_tensor("ctx_image", [257, D], f32, kind="ExternalInput"),
        "a": nc.dram_tensor("ctx_action", [64, D], f32, kind="ExternalInput"),
    }
    w_d = {
        n: nc.dram_tensor(n, [D, D], f32, kind="ExternalInput")
        for n in ["Wq", "Wk", "Wv", "Wk_ip", "Wv_ip", "Wq_a", "Wk_a", "Wv_a", "Wout"]
    }
    bout_d = nc.dram_tensor("b_out", [1, D], f32, kind="ExternalInput")
    out_d = nc.dram_tensor("out", [LQ, D], f32, kind="ExternalOutput")

    with tile.TileContext(nc) as tc, ExitStack() as ctx:
        # ---------------- persistent pools ----------------
        const = ctx.enter_context(tc.tile_pool(name="const", bufs=1))
        identity = const.tile([128, 128], bf16)
        make_identity(nc, identity)
        ones1 = const.tile([1, 128], bf16)
        nc.vector.memset(ones1, 1.0)
        bout_bf = const.tile([1, D], bf16)

        kvp = ctx.enter_context(tc.tile_pool(name="kv", bufs=1))
        kT = {b: kvp.tile([128, 8, L], bf16, name=f"kT_{b}") for b, L in BRANCHES.items()}
        # v' for even heads: [v | 1]  (PV out: o at partitions 0:64, l at 64)
        # v' for odd heads:  [1 | 0*63 | v]  (PV out: l at partition 0, o at
        # 64:128 — zero-pad keeps the matmul output base partition at 0 and
        # the l row at a 32-aligned partition for DVE)
        vE = {
            b: [kvp.tile([128, H // 2, DH + 1], bf16, name=f"vE_{b}_{ci}")
                for ci in range(len(_chunks(L)))]
            for b, L in BRANCHES.items()
        }
        vO = {
            b: [kvp.tile([128, H // 2, 128], bf16, name=f"vO_{b}_{ci}")
                for ci in range(len(_chunks(L)))]
            for b, L in BRANCHES.items()
        }
        ctxT = {b: kvp.tile([128, 8, L], bf16, name=f"ctxT_{b}") for b, L in BRANCHES.items()}

        wp = ctx.enter_context(tc.tile_pool(name="wpers", bufs=1))
        wq_bf = wp.tile([128, 8, D], bf16, name="wq_bf")
        wqa_bf = wp.tile([128, 8, D], bf16, name="wqa_bf")
        wout_bf = wp.tile([128, 8, D], bf16, name="wout_bf")

        # PSUM pools
        psum_tr = ctx.enter_context(tc.tile_pool(name="ptr", bufs=1, space="PSUM"))
        psum_mm = ctx.enter_context(tc.tile_pool(name="pmm", bufs=2, space="PSUM"))
        psum_qk = ctx.enter_context(tc.tile_pool(name="pqk", bufs=2, space="PSUM"))
        psum_pv = ctx.enter_context(tc.tile_pool(name="ppv", bufs=3, space="PSUM"))

        # ---------------- startup phase (freed before main loop) ----------------
        with ExitStack() as sctx:
            stage = sctx.enter_context(tc.tile_pool(name="stage", bufs=3))
            wkv = sctx.enter_context(tc.tile_pool(name="wkv", bufs=2))

            st = stage.tile([1, D], f32, tag="bstg")
            nc.sync.dma_start(out=st, in_=bout_d[:])
            nc.vector.tensor_copy(bout_bf, st)

            # ctx -> bf16 -> transposed ctxT
            for b, L in BRANCHES.items():
                for c0, Lc in _chunks(L):
                    st = stage.tile([128, D], f32, tag="stg")
                    nc.sync.dma_start(out=st[:Lc], in_=ctx_d[b][c0:c0 + Lc, :])
                    cb = stage.tile([128, D], bf16, tag="stgb")
                    nc.vector.tensor_copy(cb[:Lc], st[:Lc])
                    for dj in range(8):
                        pt = psum_tr.tile([128, 128], bf16)
                        nc.tensor.transpose(pt[:, :Lc], cb[:Lc, dj * 128:(dj + 1) * 128], identity[:Lc, :Lc])
                        nc.vector.tensor_copy(ctxT[b][:, dj, c0:c0 + Lc], pt[:, :Lc])

            # kv projections
            kv_specs = [
                ("Wk", "p", "k"), ("Wv", "p", "v"),
                ("Wk_ip", "i", "k"), ("Wv_ip", "i", "v"),
                ("Wk_a", "a", "k"), ("Wv_a", "a", "v"),
            ]
            for wname, b, kind in kv_specs:
                L = BRANCHES[b]
                wt = wkv.tile([128, 8, D], bf16, tag="wkv")
                for k in range(8):
                    st = stage.tile([128, D], f32, tag="stg")
                    nc.sync.dma_start(out=st, in_=w_d[wname][k * 128:(k + 1) * 128, :])
                    nc.vector.tensor_copy(wt[:, k, :], st)
                if kind == "k":
                    # kT[b][:, m, :] = (W^T ctxT)[m-chunk]
                    for m in range(8):
                        ps = psum_mm.tile([128, 512], f32, tag="ps_mm")
                        for k in range(8):
                            nc.tensor.matmul(
                                ps[:, :L],
                                lhsT=wt[:, k, m * 128:(m + 1) * 128],
                                rhs=ctxT[b][:, k, :],
                                start=(k == 0), stop=(k == 7),
                            )
                        nc.vector.tensor_copy(kT[b][:, m, :], ps[:, :L])
                else:
                    for ci, (c0, Lc) in enumerate(_chunks(L)):
                        vte = vE[b][ci]
                        vto = vO[b][ci]
                        nc.vector.memset(vte[:, :, DH:DH + 1], 1.0)
                        nc.vector.memset(vto[:, :, 0:1], 1.0)
                        nc.vector.memset(vto[:, :, 1:DH], 0.0)
                        for n in range(2):
                            ps = psum_mm.tile([128, 512], f32, tag="ps_mm")
                            for k in range(8):
                                nc.tensor.matmul(
                                    ps[:Lc],
                                    lhsT=ctxT[b][:, k, c0:c0 + Lc],
                                    rhs=wt[:, k, n * 512:(n + 1) * 512],
                                    start=(k == 0), stop=(k == 7),
                                )
                            psh = ps[:Lc].rearrange("p (h d) -> p h d", d=DH)
                            # branch gate is folded into v (NOT the ones col),
                            # so attn_head's divide yields gate * softmax @ v
                            gate = {"p": 1.0, "i": gate_i, "a": gate_a}[b]
                            # even global heads (8n+0,2,4,6) -> vE[4n..4n+3, 0:64]
                            nc.vector.tensor_scalar_mul(
                                vte[:Lc, 4 * n:4 * n + 4, 0:DH], psh[:, 0:8:2, :],
                                float(gate))
                            # odd global heads -> vO[4n..4n+3, 64:128]
                            nc.vector.tensor_scalar_mul(
                                vto[:Lc, 4 * n:4 * n + 4, DH:2 * DH], psh[:, 1:8:2, :],
                                float(gate))

            # persistent projection weights
            for wname, wt in [("Wq", wq_bf), ("Wq_a", wqa_bf), ("Wout", wout_bf)]:
                for k in range(8):
                    st = stage.tile([128, D], f32, tag="stg")
                    nc.sync.dma_start(out=st, in_=w_d[wname][k * 128:(k + 1) * 128, :])
                    nc.vector.tensor_copy(wt[:, k, :], st)

        # ---------------- steady-state q-tile loop ----------------
        qp = ctx.enter_context(tc.tile_pool(name="qtile", bufs=2))
        pp = ctx.enter_context(tc.tile_pool(name="ppool", bufs=4))
        pp5 = ctx.enter_context(tc.tile_pool(name="ppool5", bufs=5))
        pp6 = ctx.enter_context(tc.tile_pool(name="ppool6", bufs=6))

        def attn_head(qsrc, b, h, dst, base):
            """One head of one branch; writes the gate-scaled normalized
            output (gate is pre-folded into v') into dst[head-slice]; if base
            is not None, adds base[head-slice]."""
            odd = h % 2
            off = 64 * odd          # partition offset of this head's rows
            l_off = 0 if odd else 64   # partition of the denominator row
            chs = _chunks(BRANCHES[b])
            # borrow the (attention-phase-idle) projection psum pool for every
            # third head: effective PV-output depth 3+2 banks
            po_pool = psum_mm if h % 3 == 2 else psum_pv
            ps_o = po_pool.tile([128, 512], f32, tag="ps_mm" if h % 3 == 2 else "ps_o")
            for ci, (c0, Lc) in enumerate(chs):
                ps_s = psum_qk.tile([128, 512], f32, tag="ps_s")
                nc.tensor.matmul(
                    ps_s[:Lc],
                    lhsT=kT[b][off:off + 64, h // 2, c0:c0 + Lc],
                    rhs=qsrc[off:off + 64, h // 2, :],
                    start=True, stop=True,
                )
                p_sb = pp6.tile([128, 512], bf16, tag="p_sb")
                nc.scalar.activation(p_sb[:Lc], ps_s[:Lc], AF.Exp, scale=SCALE)
                vt = vO[b][ci] if odd else vE[b][ci]
                nc.tensor.matmul(
                    ps_o[0:128] if odd else ps_o[0:65],
                    lhsT=vt[:Lc, h // 2, :],
                    rhs=p_sb[:Lc],
                    start=(ci == 0), stop=(ci == len(chs) - 1),
                )
            # 1/l on the DVE (single-partition, ~0.7us), then broadcast the
            # row across this head's 64 partitions via DMA (partition dim
            # count 1, then a step-0 free dim replicating the row)
            lrow = pp.tile([65, 512], f32, tag="lrow")
            nc.vector.reciprocal(lrow[l_off:l_off + 1], ps_o[l_off:l_off + 1])
            lb = pp5.tile([128, 512], f32, tag="lb")
            ls = lrow[l_off:l_off + 1]
            lsrc = bass.AP(
                tensor=ls.tensor,
                offset=ls.offset,
                ap=[list(ls.ap[0]), [0, 64], list(ls.ap[1])],
            )
            nc.sync.dma_start(out=lb[off:off + 64], in_=lsrc)
            dslice = dst[off:off + 64, h // 2, :]
            o_ap = ps_o[off:off + 64]
            if base is None:
                nc.vector.tensor_mul(dslice, o_ap, lb[off:off + 64])
            else:
                tmp = pp.tile([128, 512], f32, tag="tmp")
                nc.vector.tensor_mul(tmp[off:off + 64], o_ap, lb[off:off + 64])
                # all-SBUF operands -> legal on the (otherwise idle) gpsimd
                nc.gpsimd.tensor_add(
                    dslice, tmp[off:off + 64], base[off:off + 64, h // 2, :])

        for t in range(NQT):
            r0 = t * QT
            # x -> bf16 -> xT
            xT = qp.tile([128, 8, QT], bf16, tag="xT")
            for ts in range(4):
                xst = qp.tile([128, D], f32, tag="xst")
                nc.sync.dma_start(out=xst, in_=x_d[r0 + ts * 128:r0 + (ts + 1) * 128, :])
                xbf = qp.tile([128, D], bf16, tag="xbf")
                nc.vector.tensor_copy(xbf, xst)
                for dj in range(8):
                    pt = psum_tr.tile([128, 128], bf16)
                    nc.tensor.transpose(pt, xbf[:, dj * 128:(dj + 1) * 128], identity)
                    nc.vector.tensor_copy(xT[:, dj, ts * 128:(ts + 1) * 128], pt)
            # qT = Wq^T @ xT
            qTt = qp.tile([128, 8, QT], bf16, tag="qTt")
            for m in range(8):
                ps = psum_mm.tile([128, 512], f32, tag="ps_mm")
                for k in range(8):
                    nc.tensor.matmul(
                        ps, lhsT=wq_bf[:, k, m * 128:(m + 1) * 128], rhs=xT[:, k, :],
                        start=(k == 0), stop=(k == 7))
                nc.scalar.copy(qTt[:, m, :], ps)
            # prompt + image attention
            acc = qp.tile([128, 8, QT], bf16, tag="acc")
            for h in range(H):
                attn_head(qTt, "p", h, acc, None)
                attn_head(qTt, "i", h, acc, acc)
            # q_a = Wq_a^T @ acc
            qaT = qp.tile([128, 8, QT], bf16, tag="qaT")
            for m in range(8):
                ps = psum_mm.tile([128, 512], f32, tag="ps_mm")
                for k in range(8):
                    nc.tensor.matmul(
                        ps, lhsT=wqa_bf[:, k, m * 128:(m + 1) * 128], rhs=acc[:, k, :],
                        start=(k == 0), stop=(k == 7))
                nc.scalar.copy(qaT[:, m, :], ps)
            # action attention
            accF = qp.tile([128, 8, QT], bf16, tag="accF")
            for h in range(H):
                attn_head(qaT, "a", h, accF, acc)
            # final projection + bias
            for ms in range(4):
                for n in range(2):
                    ps = psum_mm.tile([128, 512], f32, tag="ps_mm")
                    for k in range(8):
                        nc.tensor.matmul(
                            ps,
                            lhsT=accF[:, k, ms * 128:(ms + 1) * 128],
                            rhs=wout_bf[:, k, n * 512:(n + 1) * 512],
                            start=(k == 0), stop=False)
                    nc.tensor.matmul(
                        ps, lhsT=ones1, rhs=bout_bf[:, n * 512:(n + 1) * 512],
                        start=False, stop=True)
                    fin = qp.tile([128, 512], f32, tag="fin")
                    nc.scalar.copy(fin, ps)
                    nc.sync.dma_start(
                        out=out_d[r0 + ms * 128:r0 + (ms + 1) * 128, n * 512:(n + 1) * 512],
                        in_=fin)

    nc.finalize()
    return nc


_CACHE = {}


def _get_nc(gate_i: float, gate_a: float):
    key = (round(gate_i, 9), round(gate_a, 9))
    if key not in _CACHE:
        _CACHE[key] = _build_nc(gate_i, gate_a)
    return _CACHE[key]


def _shard_inputs(inputs):
    f = lambda a: np.ascontiguousarray(np.asarray(a), dtype=np.float32)
    weights = {n: f(inputs[n]) for n in
               ["Wq", "Wk", "Wv", "Wk_ip", "Wv_ip", "Wq_a", "Wk_a", "Wv_a", "Wout"]}
    bout = f(inputs["b_out"]).reshape(1, D)
    x = f(inputs["x"])
    cp = f(inputs["ctx_prompt"])
    ci = f(inputs["ctx_image"])
    ca = f(inputs["ctx_action"])
    in_maps = []
    for c in range(NCORES):
        m = dict(weights)
        m["b_out"] = bout
        m["x"] = x[c]
        m["ctx_prompt"] = cp[c]
        m["ctx_image"] = ci[c]
        m["ctx_action"] = ca[c]
        in_maps.append(m)
    return in_maps


def kernel(**inputs):
    from concourse.bass_utils import run_bass_kernel_spmd

    gate_i = float(np.tanh(np.float32(inputs["alpha"])) + 1.0)
    gate_a = float(np.tanh(np.float32(inputs["alpha_action"])) + 1.0)
    nc = _get_nc(gate_i, gate_a)
    in_maps = _shard_inputs(inputs)
    res = run_bass_kernel_spmd(nc, in_maps, core_ids=list(range(NCORES)))
    out = np.stack([res.results[c]["out"] for c in range(NCORES)], axis=0)
    return out.astype(np.float32)

